# revision 1
# baseline (speedup 1.0000x reference)
"""Causal single-head attention (B=4, T=4096, C=512, H=64) on 8 trn2 NeuronCores.

Sharding: core (2b+par) handles batch b and the query 128-row blocks with
parity `par` (even/odd interleave). All 8 cores run an IDENTICAL program
(SPMD) with balanced causal work; the only cross-core difference is input
data (which rows, and the causal-mask tiles).

x^T arrives column-PERMUTED: the host swaps adjacent 128-col blocks for
par=1 cores so each core's OWN query blocks sit at even positions -- the
causal structure over positions is then identical on every core, and Q
projects from strided x^T reads (no separate x_q^T load).

Column-major ("piece-phase") schedule: av piece p (q cols [512p, 512p+512))
is fully accumulated in phase p over all k-tile pairs j <= 4p+3, descending
j.  Phase p only needs x^T positions [0, 1024(p+1)) -- and only the top
1MB is NEW -- so DMA (descending 512-col chunks within each ascending
phase) always arrives just ahead of consumption.  One av piece is live at
a time, so av double-buffers in 2 banks and the scores pool gets 3 buffers.
Per unit (pair j, phase p):
    S^T = K_tile^T Q^T -> PSUM     (64-contraction, both pair slots)
    exp(S^T/sqrt(C)) -> P^T bf16   (one activation per unit)
    multiplicative causal mask on the diagonal unit (DVE, 4x bf16)
    av[65, piece] += V_k^T P_k^T   (2 bf16 matmuls; fp8 fails the 2e-2 gate)
AV trails scores by one unit; K/V/Q projections flow through a background
queue popped between units so the PE never blocks on DMA and ScalarE never
starves.  Epilogue per phase: copy av -> SBUF, 4 batched PE transposes, one
strided reciprocal + one broadcast multiply, DMA out; it overlaps the next
phase.
"""

import math

import numpy as np
import ml_dtypes

T = 4096
C = 512
H = 64
B = 4
NCORES = 8
TQ = T // 2          # own query rows per core
NJ = TQ // 128       # 16 own q blocks
NK = T // 128        # 32 k tiles
NP = NK // 2         # 16 k-tile pairs
PIECE = 512          # av piece width (f32, one PSUM bank)
NPIECES = TQ // PIECE

BF16 = ml_dtypes.bfloat16

_PROGRAM_CACHE = {}


def build_program():
    import concourse.bass as bass
    import concourse.mybir as mybir
    from concourse import bacc
    from concourse.tile import TileContext
    from concourse.masks import make_identity

    f32 = mybir.dt.float32
    bf16 = mybir.dt.bfloat16

    nc = bacc.Bacc(
        "TRN2", target_bir_lowering=False, debug=False, num_devices=NCORES
    )

    xT_d = nc.dram_tensor("xT", [C, T], bf16, kind="ExternalInput").ap()
    # packed [wq (4*64) | wkv (4*128) | msk2 (2*128)] per partition
    wpack_d = nc.dram_tensor("wpack", [128, 1024], bf16, kind="ExternalInput").ap()
    out_d = nc.dram_tensor("out", [TQ, H], f32, kind="ExternalOutput").ap()

    EXP = mybir.ActivationFunctionType.Exp
    inv_sqrt_c = 1.0 / math.sqrt(C)
    out_r = out_d.rearrange("(n p) h -> p n h", p=128)

    with TileContext(nc) as tc:
        with (
            tc.tile_pool(name="const", bufs=1) as constp,
            tc.tile_pool(name="big", bufs=1) as bigp,
            tc.tile_pool(name="stp", bufs=3, space="PSUM") as stp,
            tc.tile_pool(name="avp", bufs=1, space="PSUM") as avp,
            tc.tile_pool(name="ptp", bufs=3) as ptp,
            tc.tile_pool(name="rcpp", bufs=2) as rcpp,
        ):
            wpack_sb = constp.tile([128, 1024], bf16)
            xT_sb = bigp.tile([128, 4, T], bf16)
            xT_r = xT_d.rearrange("(a p) t -> p a t", p=128)

            wq_sb = wpack_sb[:, 0:256].rearrange("p (a h) -> p a h", a=4)
            wkv_sb = wpack_sb[:, 256:768].rearrange("p (a h) -> p a h", a=4)
            msk2_sb = wpack_sb[:, 768:1024].rearrange("p (s q) -> p s q", s=2)

            def dma_x(c0, c1):
                sl = slice(c0, c1)
                nc.sync.dma_start(out=xT_sb[:, :, sl], in_=xT_r[:, :, sl])

            # DMA: ascending 512-col chunks, matching ascending units
            nc.sync.dma_start(out=wpack_sb[:], in_=wpack_d)
            for p in range(NPIECES):
                dma_x(1024 * p, 1024 * p + 512)
                dma_x(1024 * p + 512, 1024 * p + 1024)

            ident = constp.tile([128, 128], f32)
            make_identity(nc, ident[:])
            identb = constp.tile([128, 128], bf16)
            make_identity(nc, identb[:])

            KVt = bigp.tile([128, T], bf16)   # rows 0:64 K^T, 64:128 V^T
            Qt = bigp.tile([64, TQ], bf16)
            Vn = bigp.tile([128, NK, H + 1], bf16)  # V natural + ones col
            avT = bigp.tile([65, TQ], f32)
            outsb = bigp.tile([128, NJ, H], f32)

            nc.gpsimd.memset(Vn[:, :, H : H + 1], 1.0)

            # preload the exp activation table while DMA streams in
            dummy = constp.tile([128, 1], bf16)
            nc.scalar.activation(dummy[:], ident[:, 0:1], EXP, scale=1.0)

            # warm up the PE clock (HAM releases the throttle after ~3us of
            # sustained activity) while the first DMA chunks stream in;
            # bf16 identity operands only (initialized, plain dtype path)
            warmps = stp.tile([128, 2, PIECE], f32, tag="st")
            for _ in range(24):
                nc.tensor.matmul(
                    warmps[:, 0, 0:128], identb[:, :], identb[:, :],
                    start=True, stop=True,
                )

            ps_map = {}

            def kv_mm(j, a):
                # K^T|V^T for pair j's two k tiles (positions 256j..256j+256)
                if a == 0:
                    ps_map[("kv", j)] = stp.tile(
                        [128, 2, PIECE], f32, tag="st", name="pskv"
                    )
                ps = ps_map[("kv", j)]
                sl = slice(j * 256, (j + 1) * 256)
                nc.tensor.matmul(
                    ps[:, 0, 0:256], wkv_sb[:, a, :], xT_sb[:, a, sl],
                    start=(a == 0), stop=(a == 3),
                )

            def kv_copy(j):
                nc.vector.tensor_copy(
                    KVt[:, j * 256 : (j + 1) * 256],
                    ps_map.pop(("kv", j))[:, 0, 0:256],
                )

            def vt_mm(j, s):
                # V natural for k-tile 2j+s
                if s == 0:
                    ps_map[("vt", j)] = stp.tile(
                        [128, 2, PIECE], f32, tag="st", name="psvt"
                    )
                ps = ps_map[("vt", j)]
                kk = 2 * j + s
                ksl = slice(kk * 128, (kk + 1) * 128)
                nc.tensor.matmul(
                    ps[:, 0, s * H : (s + 1) * H],
                    KVt[64:128, ksl], identb[64:128, 64:128],
                    start=True, stop=True,
                )

            def vt_copy(j):
                nc.vector.tensor_copy(
                    Vn[:, 2 * j : 2 * j + 2, 0:H], ps_map.pop(("vt", j))[:, 0, 0:128]
                )

            def qp_mm(m, a):
                # Q blocks 2m, 2m+1 (q cols [256m, 256m+256)); the blocks
                # sit at x^T positions 4m and 4m+2 (contiguous slices),
                # accumulated in separate PSUM banks (slots)
                if a == 0:
                    ps_map[("q", m)] = stp.tile(
                        [128, 2, PIECE], f32, tag="st", name="psq"
                    )
                ps = ps_map[("q", m)]
                for s in range(2):
                    lo = 512 * m + 256 * s
                    nc.tensor.matmul(
                        ps[0:64, s, 0:128], wq_sb[:, a, :],
                        xT_sb[:, a, lo : lo + 128],
                        start=(a == 0), stop=(a == 3),
                    )

            def qp_copy(m):
                ps = ps_map.pop(("q", m))
                nc.vector.tensor_copy(
                    Qt[:, m * 256 : (m + 1) * 256], ps[0:64, 0:2, 0:128]
                )

            done = {}

            def q_items(m):
                if ("q", m) in done:
                    return []
                done[("q", m)] = True
                items = [lambda a=a: qp_mm(m, a) for a in range(4)]
                items.append(lambda: qp_copy(m))
                return items

            def prereq_items(j, p):
                """Projection items needed by unit (pair j, phase p)."""
                items = []
                if ("kv", j) not in done:
                    done[("kv", j)] = True
                    items += [lambda a=a: kv_mm(j, a) for a in range(4)]
                    items.append(lambda: kv_copy(j))
                    items.append(lambda: vt_mm(j, 0))
                    items.append(lambda: vt_mm(j, 1))
                    items.append(lambda: vt_copy(j))
                return items

            def epilogue(p, av_tile):
                sl = slice(p * PIECE, (p + 1) * PIECE)
                nc.vector.tensor_copy(avT[:, sl], av_tile[:, :])
                nat = stp.tile([128, 2, PIECE], f32, tag="st")
                for jj in range(4):
                    jb = 4 * p + jj
                    nc.tensor.transpose(
                        nat[:, 0, jj * 128 : jj * 128 + 65],
                        avT[:, jb * 128 : (jb + 1) * 128],
                        ident[0:65, 0:65],
                    )
                for jj in range(4):
                    rc = rcpp.tile([128, 1], f32, tag="rc")
                    nc.vector.reciprocal(
                        rc[:], nat[:, 0, jj * 128 + 64 : jj * 128 + 65]
                    )
                    nc.vector.tensor_scalar_mul(
                        outsb[:, 4 * p + jj, :],
                        nat[:, 0, jj * 128 : jj * 128 + H], rc[:],
                    )
                nc.sync.dma_start(
                    out=out_r[:, 4 * p : 4 * p + 4, :],
                    in_=outsb[:, 4 * p : 4 * p + 4, :],
                )

            bgq = []          # (key, fn) FIFO of queued projection items
            pending = {}      # key -> items of that bundle still queued

            def pops(k):
                for _ in range(k):
                    if bgq:
                        key, fn = bgq.pop(0)
                        pending[key] -= 1
                        fn()

            def push(key, items):
                pending[key] = pending.get(key, 0) + len(items)
                bgq.extend((key, it) for it in items)

            def ensure(key):
                # drain the FIFO through this bundle so its items are
                # emitted before any consumer (emission-order safety)
                while pending.get(key, 0) > 0:
                    pops(1)

            def av_mms(prev_unit):
                ptj, jj, pp, prlo, pav = prev_unit
                for s in range(2):
                    nc.tensor.matmul(
                        pav[:, prlo:PIECE], Vn[:, 2 * jj + s, :],
                        ptj[:, s, prlo:PIECE],
                        start=(jj == 0 and s == 0),
                        stop=(jj == 4 * pp + 3 and s == 1),
                    )
                if jj == 4 * pp + 3:
                    epilogue(pp, pav)

            av_all = avp.tile([65, 2, PIECE], f32)
            prev = None  # (pt, pair j, phase p, rlo, av tile) awaiting AV
            for p in range(NPIECES):
                av = av_all[:, p % 2, :]
                # this phase's Q halves MUST be emitted before unit 0 reads
                # the full piece (emission-order race otherwise); at phase 0
                # interleave kv(0) so its chain overlaps the 2nd chunk's DMA
                ensure(("q", 2 * p))
                for it in q_items(2 * p):
                    it()
                if p == 0:
                    for it in prereq_items(0, 0):
                        it()
                ensure(("q", 2 * p + 1))
                for it in q_items(2 * p + 1):
                    it()
                # queue next phase's Q + kv projection work
                if p + 1 < NPIECES:
                    push(("q", 2 * p + 2), q_items(2 * p + 2))
                    push(("q", 2 * p + 3), q_items(2 * p + 3))
                    for j in range(4 * p + 4, 4 * p + 8):
                        push(("kv", j), prereq_items(j, p + 1))
                for j in range(0, 4 * p + 4):
                    # drain queued + inline any prereqs this unit still needs
                    ensure(("kv", j))
                    for it in prereq_items(j, p):
                        it()
                    cl0 = 128 * j
                    rlo = max(cl0 - p * PIECE, 0)
                    st = stp.tile([128, 2, PIECE], f32, tag="st")
                    for s in range(2):
                        k = 2 * j + s
                        ksl = slice(k * 128, (k + 1) * 128)
                        nc.tensor.matmul(
                            st[:, s, rlo:PIECE], KVt[0:64, ksl],
                            Qt[:, p * PIECE + rlo : (p + 1) * PIECE],
                            start=True, stop=True,
                        )
                    pt = ptp.tile([128, 2, PIECE], bf16, tag="pt")
                    nc.scalar.activation(
                        pt[:, :, rlo:PIECE], st[:, :, rlo:PIECE], EXP,
                        scale=inv_sqrt_c,
                    )
                    if j >= 4 * p:
                        # this unit's causal diagonal (q block j) lies in
                        # this piece: multiplicative mask on its 128 cols
                        nc.vector.tensor_mul(
                            pt[:, :, rlo : rlo + 128],
                            pt[:, :, rlo : rlo + 128], msk2_sb[:],
                        )
                    if prev is not None:
                        av_mms(prev)
                    prev = (pt, j, p, rlo, av)
                    pops(3)

            av_mms(prev)
            while bgq:
                bgq.pop(0)()

    nc.compile()
    return nc


def _host_inputs(x, Wq, Wk, Wv):
    """Build the 8 per-core input maps (host-side layout prep only)."""
    # msk2[kr, s, qr] multiplicative keep-mask for the diagonal position
    # pair: slot 0 (own-parity key block == q block) is triangular for both
    # cores; slot 1 is the opposite parity: fully masked for par=0 (key
    # block above the diagonal), fully kept for par=1 (below).
    tri_keep = np.triu(np.ones((128, 128), np.float32))  # [kr, qr]: qr >= kr
    wq_r = Wq.reshape(4, 128, H).transpose(1, 0, 2).reshape(128, 4 * H)
    wkv = np.concatenate([Wk, Wv], axis=1)  # [C, 128]
    wkv_r = wkv.reshape(4, 128, 2 * H).transpose(1, 0, 2).reshape(128, 4 * 2 * H)
    wpack_par = []
    for par in (0, 1):
        cols = []
        for s in (0, 1):
            if s == 0:
                keep = tri_keep
            elif par == 0:
                keep = np.zeros((128, 128), np.float32)
            else:
                keep = np.ones((128, 128), np.float32)
            cols.append(keep)
        msk2 = np.concatenate(cols, axis=1)
        wpack_par.append(
            np.concatenate([wq_r, wkv_r, msk2], axis=1).astype(BF16)
        )
    in_maps = []
    for b in range(B):
        xb = x[b]
        xT = np.ascontiguousarray(xb.T).astype(BF16)  # [C, T]
        # par=1 core: swap adjacent 128-col blocks so own blocks sit at
        # even positions
        xTsw = np.ascontiguousarray(
            xT.reshape(C, NP, 2, 128)[:, :, ::-1, :].reshape(C, T)
        )
        for par in (0, 1):
            in_maps.append(
                {
                    "xT": xT if par == 0 else xTsw,
                    "wpack": wpack_par[par],
                }
            )
    return in_maps


def kernel(x, Wq, Wk, Wv, _want_trace=False):
    from concourse.bass_utils import run_bass_kernel_spmd

    x = np.asarray(x, dtype=np.float32)
    Wq = np.asarray(Wq, dtype=np.float32)
    Wk = np.asarray(Wk, dtype=np.float32)
    Wv = np.asarray(Wv, dtype=np.float32)

    if "nc" not in _PROGRAM_CACHE:
        _PROGRAM_CACHE["nc"] = build_program()
    nc = _PROGRAM_CACHE["nc"]

    in_maps = _host_inputs(x, Wq, Wk, Wv)
    res = run_bass_kernel_spmd(
        nc, in_maps, core_ids=list(range(NCORES)), trace=_want_trace
    )

    out = np.zeros((B, T, H), np.float32)
    for b in range(B):
        for par in (0, 1):
            r = res.results[2 * b + par]["out"]
            out[b].reshape(NK, 128, H)[par::2] = np.asarray(r, np.float32).reshape(
                NJ, 128, H
            )
    if _want_trace:
        return out, res
    return out



# revision 22
# speedup vs baseline: 1.1953x; 1.1953x over previous
"""Causal single-head attention (B=4, T=4096, C=512, H=64) on 8 trn2 NeuronCores.

Sharding: core (2b+par) handles batch b and the query 128-row blocks with
parity `par` (even/odd interleave). All 8 cores run an IDENTICAL program
(SPMD) with balanced causal work; the only cross-core difference is input
data (which rows, and the causal-mask tiles).

x^T arrives column-PERMUTED: the host swaps adjacent 128-col blocks for
par=1 cores so each core's OWN query blocks sit at even positions -- the
causal structure over positions is then identical on every core, and Q
projects from strided x^T reads (no separate x_q^T load).

Column-major ("piece-phase") schedule: phase p owns q cols [512p, 512p+512)
(4 own q blocks) and accumulates them over all k-tile pairs j <= 4p+3,
ascending j.  Phase p only needs x^T positions [0, 1024(p+1)) -- and only
the top 1MB is NEW -- so DMA (ascending 512-col chunks) always arrives just
ahead of consumption.
Per unit (pair j, phase p):
    S^T = K_tile^T Q^T -> PSUM     (64-contraction, both pair slots)
    exp(S^T/sqrt(C)) -> P^T bf16   (one activation per unit)
    multiplicative causal mask on the diagonal unit (DVE, 4x bf16)
    av[qblk 128, 65] += P_k V_k    (NATURAL orientation: one matmul per
                                    (k tile, q block), free dim only 65 --
                                    half the PE rows of the transposed
                                    layout, and the output needs no
                                    transpose.  fp8 fails the 2e-2 gate.)
The 4 per-phase av accumulators ([128, 4, 65] f32) share one PSUM bank,
double-buffered across phases.  AV trails scores by one unit; K/V/Q
projections flow through a background queue popped between units so the PE
never blocks on DMA and ScalarE never starves.  Per-q-block epilogue fires
as soon as that block's accumulation stops (unit j == 4p+gi): one [128,1]
reciprocal + one [128,64] broadcast multiply, then one DMA per phase.
"""

import math

import numpy as np
import ml_dtypes

T = 4096
C = 512
H = 64
B = 4
NCORES = 8
TQ = T // 2          # own query rows per core
NJ = TQ // 128       # 16 own q blocks
NK = T // 128        # 32 k tiles
NP = NK // 2         # 16 k-tile pairs
PIECE = 512          # av piece width (f32, one PSUM bank)
NPIECES = TQ // PIECE

BF16 = ml_dtypes.bfloat16

_PROGRAM_CACHE = {}


def build_program():
    import concourse.bass as bass
    import concourse.mybir as mybir
    from concourse import bacc
    from concourse.tile import TileContext
    from concourse.masks import make_identity

    f32 = mybir.dt.float32
    bf16 = mybir.dt.bfloat16

    nc = bacc.Bacc(
        "TRN2", target_bir_lowering=False, debug=False, num_devices=NCORES
    )

    xT_d = nc.dram_tensor("xT", [C, T], bf16, kind="ExternalInput").ap()
    # packed [wq (4*64) | wkv (4*128) | msk2 (2*128)] per partition
    wpack_d = nc.dram_tensor("wpack", [128, 1024], bf16, kind="ExternalInput").ap()
    out_d = nc.dram_tensor("out", [TQ, H], f32, kind="ExternalOutput").ap()

    EXP = mybir.ActivationFunctionType.Exp
    inv_sqrt_c = 1.0 / math.sqrt(C)
    out_r = out_d.rearrange("(n p) h -> p n h", p=128)

    with TileContext(nc) as tc:
        with (
            tc.tile_pool(name="const", bufs=1) as constp,
            tc.tile_pool(name="big", bufs=1) as bigp,
            tc.tile_pool(name="stp", bufs=2, space="PSUM") as stp,
            tc.tile_pool(name="avp", bufs=2, space="PSUM") as avp,
            tc.tile_pool(name="projp", bufs=2, space="PSUM") as projp,
            tc.tile_pool(name="ptp", bufs=4) as ptp,
            tc.tile_pool(name="rcpp", bufs=2) as rcpp,
        ):
            wpack_sb = constp.tile([128, 1024], bf16)
            xT_sb = bigp.tile([128, 4, T], bf16)
            xT_r = xT_d.rearrange("(a p) t -> p a t", p=128)

            wq_sb = wpack_sb[:, 0:256].rearrange("p (a h) -> p a h", a=4)
            wkv_sb = wpack_sb[:, 256:768].rearrange("p (a h) -> p a h", a=4)
            msk2_sb = wpack_sb[:, 768:1024].rearrange("p (s q) -> p s q", s=2)

            def dma_x(c0, c1):
                sl = slice(c0, c1)
                nc.sync.dma_start(out=xT_sb[:, :, sl], in_=xT_r[:, :, sl])

            # DMA: ascending 512-col chunks, matching ascending units
            nc.sync.dma_start(out=wpack_sb[:], in_=wpack_d)
            for p in range(NPIECES):
                dma_x(1024 * p, 1024 * p + 512)
                dma_x(1024 * p + 512, 1024 * p + 1024)

            identb = constp.tile([128, 128], bf16)
            make_identity(nc, identb[:])

            KVt = bigp.tile([128, T], bf16)   # rows 0:64 K^T, 64:128 V^T
            Qt = bigp.tile([64, TQ], bf16)
            Vn = bigp.tile([128, NK, H + 1], bf16)  # V natural + ones col
            outsb = bigp.tile([128, NJ, H], f32)

            nc.gpsimd.memset(Vn[:, :, H : H + 1], 1.0)

            # preload the exp activation table while DMA streams in
            dummy = constp.tile([128, 1], bf16)
            nc.scalar.activation(dummy[:], identb[:, 0:1], EXP, scale=1.0)

            # warm up the PE clock (HAM releases the throttle after ~3us of
            # sustained activity) while the first DMA chunks stream in;
            # bf16 identity operands only (initialized, plain dtype path)
            warmps = projp.tile([128, PIECE], f32, tag="pj")
            for _ in range(24):
                nc.tensor.matmul(
                    warmps[:, 0:128], identb[:, :], identb[:, :],
                    start=True, stop=True,
                )

            ps_map = {}

            def kv_mm(j, a):
                # K^T|V^T for pair j's two k tiles (positions 256j..256j+256)
                if a == 0:
                    ps_map[("kv", j)] = projp.tile(
                        [128, PIECE], f32, tag="pj", name="pskv"
                    )
                ps = ps_map[("kv", j)]
                sl = slice(j * 256, (j + 1) * 256)
                nc.tensor.matmul(
                    ps[:, 0:256], wkv_sb[:, a, :], xT_sb[:, a, sl],
                    start=(a == 0), stop=(a == 3),
                )

            def kv_copy(j):
                nc.vector.tensor_copy(
                    KVt[:, j * 256 : (j + 1) * 256],
                    ps_map.pop(("kv", j))[:, 0:256],
                )

            def vt_mm(j, s):
                # V natural for k-tile 2j+s
                if s == 0:
                    ps_map[("vt", j)] = projp.tile(
                        [128, PIECE], f32, tag="pj", name="psvt"
                    )
                ps = ps_map[("vt", j)]
                kk = 2 * j + s
                ksl = slice(kk * 128, (kk + 1) * 128)
                nc.tensor.matmul(
                    ps[:, s * H : (s + 1) * H],
                    KVt[64:128, ksl], identb[64:128, 64:128],
                    start=True, stop=True,
                )

            def vt_copy(j):
                nc.vector.tensor_copy(
                    Vn[:, 2 * j : 2 * j + 2, 0:H], ps_map.pop(("vt", j))[:, 0:128]
                )

            def qp_mm(m, a):
                # Q blocks 2m, 2m+1 (q cols [256m, 256m+256)); the blocks
                # sit at x^T positions 4m and 4m+2 (contiguous slices)
                if a == 0:
                    ps_map[("q", m)] = projp.tile(
                        [128, PIECE], f32, tag="pj", name="psq"
                    )
                ps = ps_map[("q", m)]
                for s in range(2):
                    # both 128-col slots share one bank: a single
                    # accumulation group brackets all 8 matmuls
                    lo = 512 * m + 256 * s
                    nc.tensor.matmul(
                        ps[0:64, s * 128 : s * 128 + 128], wq_sb[:, a, :],
                        xT_sb[:, a, lo : lo + 128],
                        start=(a == 0 and s == 0), stop=(a == 3 and s == 1),
                    )

            def qp_copy(m):
                ps = ps_map.pop(("q", m))
                nc.vector.tensor_copy(
                    Qt[:, m * 256 : (m + 1) * 256], ps[0:64, 0:256]
                )

            done = {}

            def q_items(m):
                if ("q", m) in done:
                    return []
                done[("q", m)] = True
                items = [lambda a=a: qp_mm(m, a) for a in range(4)]
                items.append(lambda: qp_copy(m))
                return items

            def prereq_items(j, p):
                """Projection items needed by unit (pair j, phase p)."""
                items = []
                if ("kv", j) not in done:
                    done[("kv", j)] = True
                    items += [lambda a=a: kv_mm(j, a) for a in range(4)]
                    items.append(lambda: kv_copy(j))
                    items.append(lambda: vt_mm(j, 0))
                    items.append(lambda: vt_mm(j, 1))
                    items.append(lambda: vt_copy(j))
                return items

            def epilogue(p, gi, av_tile):
                # q block 4p+gi just finished accumulating: normalize by the
                # ones-column sum, stage to SBUF, and DMA it out immediately
                # (per-block DMAs keep the queue warm so the final block's
                # DMA only pays descriptor+transfer, not the full init).
                jb = 4 * p + gi
                rc = rcpp.tile([128, 1], f32, tag="rc")
                nc.vector.reciprocal(rc[:], av_tile[:, gi, H : H + 1])
                nc.vector.tensor_scalar_mul(
                    outsb[:, jb, :], av_tile[:, gi, 0:H], rc[:]
                )
                nc.sync.dma_start(
                    out=out_r[:, jb, :], in_=outsb[:, jb, :]
                )

            bgq = []          # (key, fn) FIFO of queued projection items
            pending = {}      # key -> items of that bundle still queued

            def pops(k):
                for _ in range(k):
                    if bgq:
                        key, fn = bgq.pop(0)
                        pending[key] -= 1
                        fn()

            def push(key, items, front=False):
                pending[key] = pending.get(key, 0) + len(items)
                wrapped = [(key, it) for it in items]
                if front:
                    # q bundles gate the next phase's first scores matmul:
                    # jump them ahead of any queued kv work
                    nq = sum(1 for kk, _ in bgq if kk[0] == "q")
                    bgq[nq:nq] = wrapped
                else:
                    bgq.extend(wrapped)

            def ensure(key):
                # drain the FIFO through this bundle so its items are
                # emitted before any consumer (emission-order safety)
                while pending.get(key, 0) > 0:
                    pops(1)

            def av_mms(prev_unit):
                # natural-orientation AV: av[qblk, 65] += P_k[qblk] @ Vn[k]
                # one matmul per (k tile, causal q block), free dim 65
                ptj, jj, pp, pav = prev_unit
                r = jj - 4 * pp  # local index of the diagonal q block
                # one accumulation group spans the whole av bank per phase:
                # start lazily zeroes the full 2KB zero region at unit 0 and
                # the single stop lands on the final diagonal matmul (only
                # one group may be open per PSUM bank at a time)
                for gi in range(max(r, 0), 4):
                    for s in range(2):
                        nc.tensor.matmul(
                            pav[:, gi, 0 : H + 1],
                            ptj[:, s, gi * 128 : (gi + 1) * 128],
                            Vn[:, 2 * jj + s, :],
                            start=(jj == 0 and gi == 0 and s == 0),
                            stop=(jj == 4 * pp + 3 and gi == 3 and s == 1),
                        )
                if jj == 4 * pp + 3:
                    # PSUM reads are only legal once the bank's group closes
                    for gi in range(4):
                        epilogue(pp, gi, pav)

            avq = []  # (pt, pair j, phase p, av tile) units awaiting AV;
            # AV trails scores by TWO units so its exp/mask (ACT/DVE) are
            # long finished when the in-order PE reaches the AV matmuls --
            # a 1-unit trail makes the PE wait out the exp latency every
            # unit.
            for p in range(NPIECES):
                # one full PSUM bank per phase (bank alignment keeps every
                # [*, gi, 0:65] accumulation region inside the bank)
                av = avp.tile([128, 4, 128], f32, tag="av")
                # this phase's Q halves MUST be emitted before unit 0 reads
                # the full piece (emission-order race otherwise); at phase 0
                # interleave kv(0) so its chain overlaps the 2nd chunk's DMA
                ensure(("q", 2 * p))
                for it in q_items(2 * p):
                    it()
                if p == 0:
                    for it in prereq_items(0, 0):
                        it()
                ensure(("q", 2 * p + 1))
                for it in q_items(2 * p + 1):
                    it()
                # queue next phase's Q + kv projection work
                if p + 1 < NPIECES:
                    push(("q", 2 * p + 2), q_items(2 * p + 2), front=True)
                    push(("q", 2 * p + 3), q_items(2 * p + 3), front=True)
                    for j in range(4 * p + 4, 4 * p + 8):
                        push(("kv", j), prereq_items(j, p + 1))
                for j in range(0, 4 * p + 4):
                    # drain queued + inline any prereqs this unit still needs
                    ensure(("kv", j))
                    for it in prereq_items(j, p):
                        it()
                    cl0 = 128 * j
                    rlo = max(cl0 - p * PIECE, 0)
                    st = stp.tile([128, 2, PIECE], f32, tag="st")
                    for s in range(2):
                        k = 2 * j + s
                        ksl = slice(k * 128, (k + 1) * 128)
                        nc.tensor.matmul(
                            st[:, s, rlo:PIECE], KVt[0:64, ksl],
                            Qt[:, p * PIECE + rlo : (p + 1) * PIECE],
                            start=True, stop=True,
                        )
                    pt = ptp.tile([128, 2, PIECE], bf16, tag="pt")
                    nc.scalar.activation(
                        pt[:, :, rlo:PIECE], st[:, :, rlo:PIECE], EXP,
                        scale=inv_sqrt_c,
                    )
                    if j >= 4 * p:
                        # this unit's causal diagonal (q block j) lies in
                        # this piece: multiplicative mask on its 128 cols
                        nc.vector.tensor_mul(
                            pt[:, :, rlo : rlo + 128],
                            pt[:, :, rlo : rlo + 128], msk2_sb[:],
                        )
                    avq.append((pt, j, p, av))
                    if len(avq) > 2:
                        av_mms(avq.pop(0))
                    pops(4)

            while avq:
                av_mms(avq.pop(0))
            while bgq:
                bgq.pop(0)()

    nc.compile()
    return nc


def _host_inputs(x, Wq, Wk, Wv):
    """Build the 8 per-core input maps (host-side layout prep only)."""
    # msk2[kr, s, qr] multiplicative keep-mask for the diagonal position
    # pair: slot 0 (own-parity key block == q block) is triangular for both
    # cores; slot 1 is the opposite parity: fully masked for par=0 (key
    # block above the diagonal), fully kept for par=1 (below).
    tri_keep = np.triu(np.ones((128, 128), np.float32))  # [kr, qr]: qr >= kr
    wq_r = Wq.reshape(4, 128, H).transpose(1, 0, 2).reshape(128, 4 * H)
    wkv = np.concatenate([Wk, Wv], axis=1)  # [C, 128]
    wkv_r = wkv.reshape(4, 128, 2 * H).transpose(1, 0, 2).reshape(128, 4 * 2 * H)
    wpack_par = []
    for par in (0, 1):
        cols = []
        for s in (0, 1):
            if s == 0:
                keep = tri_keep
            elif par == 0:
                keep = np.zeros((128, 128), np.float32)
            else:
                keep = np.ones((128, 128), np.float32)
            cols.append(keep)
        msk2 = np.concatenate(cols, axis=1)
        wpack_par.append(
            np.concatenate([wq_r, wkv_r, msk2], axis=1).astype(BF16)
        )
    in_maps = []
    for b in range(B):
        xb = x[b]
        xT = np.ascontiguousarray(xb.T).astype(BF16)  # [C, T]
        # par=1 core: swap adjacent 128-col blocks so own blocks sit at
        # even positions
        xTsw = np.ascontiguousarray(
            xT.reshape(C, NP, 2, 128)[:, :, ::-1, :].reshape(C, T)
        )
        for par in (0, 1):
            in_maps.append(
                {
                    "xT": xT if par == 0 else xTsw,
                    "wpack": wpack_par[par],
                }
            )
    return in_maps


def kernel(x, Wq, Wk, Wv, _want_trace=False):
    from concourse.bass_utils import run_bass_kernel_spmd

    x = np.asarray(x, dtype=np.float32)
    Wq = np.asarray(Wq, dtype=np.float32)
    Wk = np.asarray(Wk, dtype=np.float32)
    Wv = np.asarray(Wv, dtype=np.float32)

    if "nc" not in _PROGRAM_CACHE:
        _PROGRAM_CACHE["nc"] = build_program()
    nc = _PROGRAM_CACHE["nc"]

    in_maps = _host_inputs(x, Wq, Wk, Wv)
    res = run_bass_kernel_spmd(
        nc, in_maps, core_ids=list(range(NCORES)), trace=_want_trace
    )

    out = np.zeros((B, T, H), np.float32)
    for b in range(B):
        for par in (0, 1):
            r = res.results[2 * b + par]["out"]
            out[b].reshape(NK, 128, H)[par::2] = np.asarray(r, np.float32).reshape(
                NJ, 128, H
            )
    if _want_trace:
        return out, res
    return out



# revision 58
# speedup vs baseline: 1.2837x; 1.0740x over previous
"""Causal single-head attention (B=4, T=4096, C=512, H=64) on 8 trn2 NeuronCores.

Sharding: core (2b+par) handles batch b and the query 128-row blocks with
parity `par` (even/odd interleave). All 8 cores run an IDENTICAL program
(SPMD) with balanced causal work; the only cross-core difference is input
data (which rows, and the causal-mask tiles).

x^T arrives column-PERMUTED: the host swaps adjacent 128-col blocks for
par=1 cores so each core's OWN query blocks sit at even positions -- the
causal structure over positions is then identical on every core, and Q
projects from strided x^T reads (no separate x_q^T load).

Column-major ("piece-phase") schedule: phase p owns q cols [512p, 512p+512)
(4 own q blocks) and accumulates them over all k-tile pairs j <= 4p+3,
ascending j.  Phase p only needs x^T positions [0, 1024(p+1)) -- and only
the top 1MB is NEW -- so DMA (ascending 512-col chunks) always arrives just
ahead of consumption.
Per unit (pair j, phase p):
    S^T = K_tile^T Q^T -> PSUM     (64-contraction, both pair slots)
    exp(S^T/sqrt(C)) -> P^T bf16   (one activation per unit)
    multiplicative causal mask on the diagonal unit (DVE, 4x bf16)
    av[qblk 128, 65] += P_k V_k    (NATURAL orientation: one matmul per
                                    (k tile, q block), free dim only 65 --
                                    half the PE rows of the transposed
                                    layout, and the output needs no
                                    transpose.  fp8 fails the 2e-2 gate.)
The 4 per-phase av accumulators ([128, 4, 65] f32) share one PSUM bank,
double-buffered across phases.  AV trails scores by one unit; K/V/Q
projections flow through a background queue popped between units so the PE
never blocks on DMA and ScalarE never starves.  Per-q-block epilogue fires
as soon as that block's accumulation stops (unit j == 4p+gi): one [128,1]
reciprocal + one [128,64] broadcast multiply, then one DMA per phase.
"""

import math

import numpy as np
import ml_dtypes

T = 4096
C = 512
H = 64
B = 4
NCORES = 8
TQ = T // 2          # own query rows per core
NJ = TQ // 128       # 16 own q blocks
NK = T // 128        # 32 k tiles
NP = NK // 2         # 16 k-tile pairs
PIECE = 512          # av piece width (f32, one PSUM bank)
NPIECES = TQ // PIECE

BF16 = ml_dtypes.bfloat16

_PROGRAM_CACHE = {}


def _EXP_ENGINE(p, j):
    # which engine computes exp for unit (p, j): "act" exact, "dve" fast-exp
    # (alternating units in phases >= 1, except each phase's final diagonal;
    # chosen by TimelineSim sweep)
    if p >= 1 and j % 2 == 1 and j != 4 * p + 3:
        return "dve"
    return "act"


# engine for PSUM->SBUF projection copies: "dve" or "act" per kind
# (kv copies ride the ACT engine to offset the DVE's exp share)
_COPY_ENGINE = {"kv": "act", "q": "dve", "vn": "dve"}


def build_program():
    import concourse.bass as bass
    import concourse.mybir as mybir
    from concourse import bacc
    from concourse.tile import TileContext
    from concourse.masks import make_identity

    f32 = mybir.dt.float32
    bf16 = mybir.dt.bfloat16

    nc = bacc.Bacc(
        "TRN2", target_bir_lowering=False, debug=False, num_devices=NCORES
    )

    xT_d = nc.dram_tensor("xT", [C, T], bf16, kind="ExternalInput").ap()
    # packed [wq (4*64) | wkv (4*128) | msk2 (2*128)] per partition
    wpack_d = nc.dram_tensor("wpack", [128, 1024], bf16, kind="ExternalInput").ap()
    out_d = nc.dram_tensor("out", [TQ, H], f32, kind="ExternalOutput").ap()

    EXP = mybir.ActivationFunctionType.Exp
    inv_sqrt_c = 1.0 / math.sqrt(C)
    out_r = out_d.rearrange("(n p) h -> p n h", p=128)

    # Schraudolph fast-exp constants: bf16 bits of exp(s/sqrt(C)) are
    # approximated by int16(trunc(A*s + B)); the linear-in-mantissa error
    # (~1.7% std) is multiplicative and its mean cancels exactly in the
    # softmax ratio.  Used on the Pool/DVE engines for a subset of units to
    # offload the ACT engine (the exp throughput bottleneck).
    SCH_A = 128.0 * math.log2(math.e) / math.sqrt(C)
    SCH_B = 16248.0

    # GPSIMD/Pool cannot read PSUM on real HW, so the fast-exp offload goes
    # to the DVE; the DVE's projection copies move to the ACT engine (which
    # CAN read PSUM) to compensate.  Assignment knobs are module globals so
    # the schedule can be tuned by simulation sweep.
    def exp_engine(p, j):
        return _EXP_ENGINE(p, j)

    with TileContext(nc) as tc:
        with (
            tc.tile_pool(name="const", bufs=1) as constp,
            tc.tile_pool(name="big", bufs=1) as bigp,
            tc.tile_pool(name="stp", bufs=4, space="PSUM") as stp,
            tc.tile_pool(name="avp", bufs=2, space="PSUM") as avp,
            tc.tile_pool(name="projp", bufs=2, space="PSUM") as projp,
            tc.tile_pool(name="ptp", bufs=4) as ptp,
            tc.tile_pool(name="rcpp", bufs=2) as rcpp,
        ):
            wpack_sb = constp.tile([128, 1024], bf16)
            xT_sb = bigp.tile([128, 4, T], bf16)
            xT_r = xT_d.rearrange("(a p) t -> p a t", p=128)

            wq_sb = wpack_sb[:, 0:256].rearrange("p (a h) -> p a h", a=4)
            wkv_sb = wpack_sb[:, 256:768].rearrange("p (a h) -> p a h", a=4)
            msk2_sb = wpack_sb[:, 768:1024].rearrange("p (s q) -> p s q", s=2)

            def dma_x(c0, c1):
                sl = slice(c0, c1)
                nc.sync.dma_start(out=xT_sb[:, :, sl], in_=xT_r[:, :, sl])

            # DMA: ascending 512-col chunks, matching ascending units
            nc.sync.dma_start(out=wpack_sb[:], in_=wpack_d)
            for p in range(NPIECES):
                dma_x(1024 * p, 1024 * p + 512)
                dma_x(1024 * p + 512, 1024 * p + 1024)

            identb = constp.tile([128, 128], bf16)
            make_identity(nc, identb[:])

            KVt = bigp.tile([128, T], bf16)   # rows 0:64 K^T, 64:128 V^T
            Qt = bigp.tile([64, TQ], bf16)
            Vn = bigp.tile([128, NK, H + 1], bf16)  # V natural + ones col
            outsb = bigp.tile([128, NJ, H], f32)

            nc.gpsimd.memset(Vn[:, :, H : H + 1], 1.0)

            # preload the exp activation table while DMA streams in
            dummy = constp.tile([128, 1], bf16)
            nc.scalar.activation(dummy[:], identb[:, 0:1], EXP, scale=1.0)

            # warm up the PE clock (HAM releases the throttle after ~3us of
            # sustained activity) while the first DMA chunks stream in;
            # bf16 identity operands only (initialized, plain dtype path)
            warmps = projp.tile([128, PIECE], f32, tag="pj")
            for _ in range(24):
                nc.tensor.matmul(
                    warmps[:, 0:128], identb[:, :], identb[:, :],
                    start=True, stop=True,
                )

            ps_map = {}

            def kv_mm(j, a):
                # K^T|V^T for pair j's two k tiles (positions 256j..256j+256)
                if a == 0:
                    ps_map[("kv", j)] = projp.tile(
                        [128, PIECE], f32, tag="pj", name="pskv"
                    )
                ps = ps_map[("kv", j)]
                sl = slice(j * 256, (j + 1) * 256)
                nc.tensor.matmul(
                    ps[:, 0:256], wkv_sb[:, a, :], xT_sb[:, a, sl],
                    start=(a == 0), stop=(a == 3),
                )

            def proj_copy(kind, out_ap, in_ap):
                if _COPY_ENGINE[kind] == "act":
                    nc.scalar.copy(out_ap, in_ap)
                else:
                    nc.vector.tensor_copy(out_ap, in_ap)

            def kv_copy(j):
                proj_copy(
                    "kv",
                    KVt[:, j * 256 : (j + 1) * 256],
                    ps_map.pop(("kv", j))[:, 0:256],
                )

            def vt_mm(j, s):
                # V natural for k-tile 2j+s
                if s == 0:
                    ps_map[("vt", j)] = projp.tile(
                        [128, PIECE], f32, tag="pj", name="psvt"
                    )
                ps = ps_map[("vt", j)]
                kk = 2 * j + s
                ksl = slice(kk * 128, (kk + 1) * 128)
                nc.tensor.matmul(
                    ps[:, s * H : (s + 1) * H],
                    KVt[64:128, ksl], identb[64:128, 64:128],
                    start=True, stop=True,
                )

            def vt_copy(j):
                proj_copy(
                    "vn",
                    Vn[:, 2 * j : 2 * j + 2, 0:H],
                    ps_map.pop(("vt", j))[:, 0:128],
                )

            def qp_mm(m, a):
                # Q blocks 2m, 2m+1 (q cols [256m, 256m+256)); the blocks
                # sit at x^T positions 4m and 4m+2 (contiguous slices)
                if a == 0:
                    ps_map[("q", m)] = projp.tile(
                        [128, PIECE], f32, tag="pj", name="psq"
                    )
                ps = ps_map[("q", m)]
                for s in range(2):
                    # both 128-col slots share one bank: a single
                    # accumulation group brackets all 8 matmuls
                    lo = 512 * m + 256 * s
                    nc.tensor.matmul(
                        ps[0:64, s * 128 : s * 128 + 128], wq_sb[:, a, :],
                        xT_sb[:, a, lo : lo + 128],
                        start=(a == 0 and s == 0), stop=(a == 3 and s == 1),
                    )

            def qp_copy(m):
                ps = ps_map.pop(("q", m))
                proj_copy("q", Qt[:, m * 256 : (m + 1) * 256], ps[0:64, 0:256])

            done = {}

            def q_items(m):
                if ("q", m) in done:
                    return []
                done[("q", m)] = True
                items = [lambda a=a: qp_mm(m, a) for a in range(4)]
                items.append(lambda: qp_copy(m))
                return items

            def prereq_items(j, p):
                """Projection items needed by unit (pair j, phase p)."""
                items = []
                if ("kv", j) not in done:
                    done[("kv", j)] = True
                    items += [lambda a=a: kv_mm(j, a) for a in range(4)]
                    items.append(lambda: kv_copy(j))
                    items.append(lambda: vt_mm(j, 0))
                    items.append(lambda: vt_mm(j, 1))
                    items.append(lambda: vt_copy(j))
                return items

            def epilogue(p, gi, bank):
                # q block 4p+gi just finished accumulating: normalize by the
                # ones-column sum, stage to SBUF, and DMA it out immediately
                # (per-block DMAs keep the queue warm so the final block's
                # DMA only pays descriptor+transfer, not the full init).
                jb = 4 * p + gi
                sl = gi if gi < 3 else 0
                rc = rcpp.tile([128, 1], f32, tag="rc")
                nc.vector.reciprocal(rc[:], bank[:, sl, H : H + 1])
                nc.vector.tensor_scalar_mul(
                    outsb[:, jb, :], bank[:, sl, 0:H], rc[:]
                )
                if p == NPIECES - 1 and gi == 3:
                    # last block rides the second HWDGE queue (ACT engine is
                    # done with exps) so the final two DMAs overlap
                    nc.scalar.dma_start(
                        out=out_r[:, jb, :], in_=outsb[:, jb, :]
                    )
                else:
                    nc.sync.dma_start(
                        out=out_r[:, jb, :], in_=outsb[:, jb, :]
                    )

            bgq = []          # (key, fn) FIFO of queued projection items
            pending = {}      # key -> items of that bundle still queued

            def pops(k):
                for _ in range(k):
                    if bgq:
                        key, fn = bgq.pop(0)
                        pending[key] -= 1
                        fn()

            def push(key, items, front=False):
                pending[key] = pending.get(key, 0) + len(items)
                wrapped = [(key, it) for it in items]
                if front:
                    # q bundles gate the next phase's first scores matmul:
                    # jump them ahead of any queued kv work
                    nq = sum(1 for kk, _ in bgq if kk[0] == "q")
                    bgq[nq:nq] = wrapped
                else:
                    bgq.extend(wrapped)

            def ensure(key):
                # drain the FIFO through this bundle so its items are
                # emitted before any consumer (emission-order safety)
                while pending.get(key, 0) > 0:
                    pops(1)

            def av_mms(prev_unit):
                # natural-orientation AV: av[qblk, 65] += P_k[qblk] @ Vn[k]
                # one matmul per (k tile, causal q block), free dim 65
                ptj, jj, pp, pav = prev_unit
                r = jj - 4 * pp  # local index of the diagonal q block
                # only one accumulation group may be open per PSUM bank, and
                # reads are only legal after the group's stop.  q blocks
                # {0,1} share one bank (group closes at unit 4p+1, so their
                # epilogues fire mid-phase) and {2,3} the other (closes at
                # the final unit).  start lazily zeroes the whole bank.
                pav012, pav3 = pav
                for gi in range(max(r, 0), 4):
                    bank, sl = (pav012, gi) if gi < 3 else (pav3, 0)
                    for s in range(2):
                        nc.tensor.matmul(
                            bank[:, sl, 0 : H + 1],
                            ptj[:, s, gi * 128 : (gi + 1) * 128],
                            Vn[:, 2 * jj + s, :],
                            start=(jj == 0 and gi in (0, 3) and s == 0),
                            stop=(s == 1 and jj == 4 * pp +
                                  (2 if gi < 3 else 3) and gi in (2, 3)),
                        )
                if jj == 4 * pp + 2:
                    # bank A (q blocks 0..2) closed: epilogues fire with one
                    # unit still to go, keeping only block 3 in the tail
                    for gg in range(3):
                        epilogue(pp, gg, pav012)
                elif jj == 4 * pp + 3:
                    epilogue(pp, 3, pav3)

            avq = []  # (pt, pair j, phase p, av tile) units awaiting AV;
            # AV trails scores by TWO units so its exp/mask (ACT/DVE) are
            # long finished when the in-order PE reaches the AV matmuls --
            # a 1-unit trail makes the PE wait out the exp latency every
            # unit.
            for p in range(NPIECES):
                # two full PSUM banks per phase (bank alignment keeps every
                # [*, gi%2, 0:65] accumulation region inside its bank):
                # q blocks {0,1} in one, {2,3} in the other
                av012 = avp.tile([128, 4, 128], f32, tag="av", name="av012")
                av3 = avp.tile([128, 4, 128], f32, tag="av", name="av3")
                av = (av012, av3)
                # this phase's Q halves MUST be emitted before unit 0 reads
                # the full piece (emission-order race otherwise); at phase 0
                # interleave kv(0) so its chain overlaps the 2nd chunk's DMA
                ensure(("q", 2 * p))
                for it in q_items(2 * p):
                    it()
                if p == 0:
                    for it in prereq_items(0, 0):
                        it()
                ensure(("q", 2 * p + 1))
                for it in q_items(2 * p + 1):
                    it()

                # queue next phase's Q + kv projection work
                if p + 1 < NPIECES:
                    push(("q", 2 * p + 2), q_items(2 * p + 2), front=True)
                    push(("q", 2 * p + 3), q_items(2 * p + 3), front=True)
                    for j in range(4 * p + 4, 4 * p + 8):
                        push(("kv", j), prereq_items(j, p + 1))
                for j in range(0, 4 * p + 4):
                    ensure(("kv", j))
                    for it in prereq_items(j, p):
                        it()
                    cl0 = 128 * j
                    rlo = max(cl0 - p * PIECE, 0)
                    # per-slot single-bank score tiles + per-slot exp: four
                    # half-units in flight in the same 4 PSUM banks, so the
                    # scores->exp->slot-recycle latency loop overlaps twice
                    # as deep and the exp engines can run concurrently
                    pt = ptp.tile([128, 2, PIECE], bf16, tag="pt")
                    eng = exp_engine(p, j)
                    for s in range(2):
                        k = 2 * j + s
                        ksl = slice(k * 128, (k + 1) * 128)
                        stx = stp.tile([128, PIECE], f32, tag="st")
                        nc.tensor.matmul(
                            stx[:, rlo:PIECE], KVt[0:64, ksl],
                            Qt[:, p * PIECE + rlo : (p + 1) * PIECE],
                            start=True, stop=True,
                        )
                        if eng == "act":
                            nc.scalar.activation(
                                pt[:, s, rlo:PIECE], stx[:, rlo:PIECE], EXP,
                                scale=inv_sqrt_c,
                            )
                        else:
                            ve = nc.gpsimd if eng == "pool" else nc.vector
                            ve.tensor_scalar(
                                pt[:, s, rlo:PIECE].bitcast(mybir.dt.int16),
                                stx[:, rlo:PIECE],
                                SCH_A, SCH_B,
                                mybir.AluOpType.mult, mybir.AluOpType.add,
                            )
                    if j >= 4 * p:
                        # this unit's causal diagonal (q block j) lies in
                        # this piece: multiplicative mask on its 128 cols
                        nc.vector.tensor_mul(
                            pt[:, :, rlo : rlo + 128],
                            pt[:, :, rlo : rlo + 128], msk2_sb[:],
                        )
                    avq.append((pt, j, p, av))
                    if len(avq) > 2:
                        av_mms(avq.pop(0))
                    pops(4)

            while avq:
                av_mms(avq.pop(0))
            while bgq:
                bgq.pop(0)()

    nc.compile()
    return nc


def _host_inputs(x, Wq, Wk, Wv):
    """Build the 8 per-core input maps (host-side layout prep only)."""
    # msk2[kr, s, qr] multiplicative keep-mask for the diagonal position
    # pair: slot 0 (own-parity key block == q block) is triangular for both
    # cores; slot 1 is the opposite parity: fully masked for par=0 (key
    # block above the diagonal), fully kept for par=1 (below).
    tri_keep = np.triu(np.ones((128, 128), np.float32))  # [kr, qr]: qr >= kr
    wq_r = Wq.reshape(4, 128, H).transpose(1, 0, 2).reshape(128, 4 * H)
    wkv = np.concatenate([Wk, Wv], axis=1)  # [C, 128]
    wkv_r = wkv.reshape(4, 128, 2 * H).transpose(1, 0, 2).reshape(128, 4 * 2 * H)
    wpack_par = []
    for par in (0, 1):
        cols = []
        for s in (0, 1):
            if s == 0:
                keep = tri_keep
            elif par == 0:
                keep = np.zeros((128, 128), np.float32)
            else:
                keep = np.ones((128, 128), np.float32)
            cols.append(keep)
        msk2 = np.concatenate(cols, axis=1)
        wpack_par.append(
            np.concatenate([wq_r, wkv_r, msk2], axis=1).astype(BF16)
        )
    in_maps = []
    for b in range(B):
        xb = x[b]
        xT = np.ascontiguousarray(xb.T).astype(BF16)  # [C, T]
        # par=1 core: swap adjacent 128-col blocks so own blocks sit at
        # even positions
        xTsw = np.ascontiguousarray(
            xT.reshape(C, NP, 2, 128)[:, :, ::-1, :].reshape(C, T)
        )
        for par in (0, 1):
            in_maps.append(
                {
                    "xT": xT if par == 0 else xTsw,
                    "wpack": wpack_par[par],
                }
            )
    return in_maps


def kernel(x, Wq, Wk, Wv, _want_trace=False):
    from concourse.bass_utils import run_bass_kernel_spmd

    x = np.asarray(x, dtype=np.float32)
    Wq = np.asarray(Wq, dtype=np.float32)
    Wk = np.asarray(Wk, dtype=np.float32)
    Wv = np.asarray(Wv, dtype=np.float32)

    if "nc" not in _PROGRAM_CACHE:
        _PROGRAM_CACHE["nc"] = build_program()
    nc = _PROGRAM_CACHE["nc"]

    in_maps = _host_inputs(x, Wq, Wk, Wv)
    res = run_bass_kernel_spmd(
        nc, in_maps, core_ids=list(range(NCORES)), trace=_want_trace
    )

    out = np.zeros((B, T, H), np.float32)
    for b in range(B):
        for par in (0, 1):
            r = res.results[2 * b + par]["out"]
            out[b].reshape(NK, 128, H)[par::2] = np.asarray(r, np.float32).reshape(
                NJ, 128, H
            )
    if _want_trace:
        return out, res
    return out



# revision 69
# speedup vs baseline: 1.3011x; 1.0135x over previous
"""Causal single-head attention (B=4, T=4096, C=512, H=64) on 8 trn2 NeuronCores.

Sharding: core (2b+par) handles batch b and the query 128-row blocks with
parity `par` (even/odd interleave). All 8 cores run an IDENTICAL program
(SPMD) with balanced causal work; the only cross-core difference is input
data (which rows, and the causal-mask tiles).

x^T arrives column-PERMUTED: the host swaps adjacent 128-col blocks for
par=1 cores so each core's OWN query blocks sit at even positions -- the
causal structure over positions is then identical on every core, and Q
projects from strided x^T reads (no separate x_q^T load).

Column-major ("piece-phase") schedule: phase p owns q cols [512p, 512p+512)
(4 own q blocks) and accumulates them over all k-tile pairs j <= 4p+3,
ascending j.  Phase p only needs x^T positions [0, 1024(p+1)) -- and only
the top 1MB is NEW -- so DMA (ascending 512-col chunks) always arrives just
ahead of consumption.
Per unit (pair j, phase p):
    S^T = K_tile^T Q^T -> PSUM     (64-contraction, both pair slots)
    exp(S^T/sqrt(C)) -> P^T bf16   (one activation per unit)
    multiplicative causal mask on the diagonal unit (DVE, 4x bf16)
    av[qblk 128, 65] += P_k V_k    (NATURAL orientation: one matmul per
                                    (k tile, q block), free dim only 65 --
                                    half the PE rows of the transposed
                                    layout, and the output needs no
                                    transpose.  fp8 fails the 2e-2 gate.)
The 4 per-phase av accumulators ([128, 4, 65] f32) share one PSUM bank,
double-buffered across phases.  AV trails scores by one unit; K/V/Q
projections flow through a background queue popped between units so the PE
never blocks on DMA and ScalarE never starves.  Per-q-block epilogue fires
as soon as that block's accumulation stops (unit j == 4p+gi): one [128,1]
reciprocal + one [128,64] broadcast multiply, then one DMA per phase.
"""

import math

import numpy as np
import ml_dtypes

T = 4096
C = 512
H = 64
B = 4
NCORES = 8
TQ = T // 2          # own query rows per core
NJ = TQ // 128       # 16 own q blocks
NK = T // 128        # 32 k tiles
NP = NK // 2         # 16 k-tile pairs
PIECE = 512          # av piece width (f32, one PSUM bank)
NPIECES = TQ // PIECE

BF16 = ml_dtypes.bfloat16

_PROGRAM_CACHE = {}


def _EXP_ENGINE(p, j):
    # which engine computes exp for unit (p, j): "act" exact, "dve" fast-exp
    # (alternating units in phases >= 1, except each phase's final diagonal;
    # chosen by TimelineSim sweep)
    if p >= 1 and j % 2 == 1 and j != 4 * p + 3:
        return "dve"
    return "act"


# engine for PSUM->SBUF projection copies: "dve" or "act" per kind
# (kv copies ride the ACT engine to offset the DVE's exp share)
_COPY_ENGINE = {"kv": "act", "q": "dve", "vn": "dve"}


def build_program():
    import concourse.bass as bass
    import concourse.mybir as mybir
    from concourse import bacc
    from concourse.tile import TileContext
    from concourse.masks import make_identity

    f32 = mybir.dt.float32
    bf16 = mybir.dt.bfloat16

    nc = bacc.Bacc(
        "TRN2", target_bir_lowering=False, debug=False, num_devices=NCORES
    )

    xT_d = nc.dram_tensor("xT", [C, T], bf16, kind="ExternalInput").ap()
    # packed [wq (4*64) | wkv (4*128) | msk2 (2*128)] per partition
    wpack_d = nc.dram_tensor("wpack", [128, 1024], bf16, kind="ExternalInput").ap()
    out_d = nc.dram_tensor("out", [TQ, H], f32, kind="ExternalOutput").ap()

    EXP = mybir.ActivationFunctionType.Exp
    inv_sqrt_c = 1.0 / math.sqrt(C)
    out_r = out_d.rearrange("(n p) h -> p n h", p=128)

    # Schraudolph fast-exp constants: bf16 bits of exp(s/sqrt(C)) are
    # approximated by int16(trunc(A*s + B)); the linear-in-mantissa error
    # (~1.7% std) is multiplicative and its mean cancels exactly in the
    # softmax ratio.  Used on the Pool/DVE engines for a subset of units to
    # offload the ACT engine (the exp throughput bottleneck).
    SCH_A = 128.0 * math.log2(math.e) / math.sqrt(C)
    SCH_B = 16248.0

    # GPSIMD/Pool cannot read PSUM on real HW, so the fast-exp offload goes
    # to the DVE; the DVE's projection copies move to the ACT engine (which
    # CAN read PSUM) to compensate.  Assignment knobs are module globals so
    # the schedule can be tuned by simulation sweep.
    def exp_engine(p, j):
        return _EXP_ENGINE(p, j)

    with TileContext(nc) as tc:
        with (
            tc.tile_pool(name="const", bufs=1) as constp,
            tc.tile_pool(name="big", bufs=1) as bigp,
            tc.tile_pool(name="stp", bufs=4, space="PSUM") as stp,
            tc.tile_pool(name="avp", bufs=2, space="PSUM") as avp,
            tc.tile_pool(name="projp", bufs=2, space="PSUM") as projp,
            tc.tile_pool(name="ptp", bufs=6) as ptp,
            tc.tile_pool(name="rcpp", bufs=2) as rcpp,
        ):
            wpack_sb = constp.tile([128, 1024], bf16)
            xT_sb = bigp.tile([128, 4, T], bf16)
            xT_r = xT_d.rearrange("(a p) t -> p a t", p=128)

            wq_sb = wpack_sb[:, 0:256].rearrange("p (a h) -> p a h", a=4)
            wkv_sb = wpack_sb[:, 256:768].rearrange("p (a h) -> p a h", a=4)
            msk2_sb = wpack_sb[:, 768:1024].rearrange("p (s q) -> p s q", s=2)

            def dma_x(c0, c1):
                sl = slice(c0, c1)
                nc.sync.dma_start(out=xT_sb[:, :, sl], in_=xT_r[:, :, sl])

            # DMA: ascending 512-col chunks, matching ascending units
            nc.sync.dma_start(out=wpack_sb[:], in_=wpack_d)
            for p in range(NPIECES):
                dma_x(1024 * p, 1024 * p + 512)
                dma_x(1024 * p + 512, 1024 * p + 1024)

            identb = constp.tile([128, 128], bf16)
            make_identity(nc, identb[:])

            KVt = bigp.tile([128, T], bf16)   # rows 0:64 K^T, 64:128 V^T
            Qt = bigp.tile([64, TQ], bf16)
            Vn = bigp.tile([128, NK, H + 1], bf16)  # V natural + ones col
            outsb = bigp.tile([128, NJ, H], f32)

            nc.gpsimd.memset(Vn[:, :, H : H + 1], 1.0)

            # preload the exp activation table while DMA streams in
            dummy = constp.tile([128, 1], bf16)
            nc.scalar.activation(dummy[:], identb[:, 0:1], EXP, scale=1.0)

            # warm up the PE clock (HAM releases the throttle after ~3us of
            # sustained activity) while the first DMA chunks stream in;
            # bf16 identity operands only (initialized, plain dtype path)
            warmps = projp.tile([128, PIECE], f32, tag="pj")
            for _ in range(24):
                nc.tensor.matmul(
                    warmps[:, 0:128], identb[:, :], identb[:, :],
                    start=True, stop=True,
                )

            ps_map = {}

            def kv_mm(j, a):
                # K^T|V^T for pair j's two k tiles (positions 256j..256j+256)
                if a == 0:
                    ps_map[("kv", j)] = projp.tile(
                        [128, PIECE], f32, tag="pj", name="pskv"
                    )
                ps = ps_map[("kv", j)]
                sl = slice(j * 256, (j + 1) * 256)
                nc.tensor.matmul(
                    ps[:, 0:256], wkv_sb[:, a, :], xT_sb[:, a, sl],
                    start=(a == 0), stop=(a == 3),
                )

            def proj_copy(kind, out_ap, in_ap):
                if _COPY_ENGINE[kind] == "act":
                    nc.scalar.copy(out_ap, in_ap)
                else:
                    nc.vector.tensor_copy(out_ap, in_ap)

            def kv_copy(j):
                proj_copy(
                    "kv",
                    KVt[:, j * 256 : (j + 1) * 256],
                    ps_map.pop(("kv", j))[:, 0:256],
                )

            def vt_mm(j, s):
                # V natural for k-tile 2j+s
                if s == 0:
                    ps_map[("vt", j)] = projp.tile(
                        [128, PIECE], f32, tag="pj", name="psvt"
                    )
                ps = ps_map[("vt", j)]
                kk = 2 * j + s
                ksl = slice(kk * 128, (kk + 1) * 128)
                nc.tensor.matmul(
                    ps[:, s * H : (s + 1) * H],
                    KVt[64:128, ksl], identb[64:128, 64:128],
                    start=True, stop=True,
                )

            def vt_copy(j):
                proj_copy(
                    "vn",
                    Vn[:, 2 * j : 2 * j + 2, 0:H],
                    ps_map.pop(("vt", j))[:, 0:128],
                )

            def qp_mm(m, a):
                # Q blocks 2m, 2m+1 (q cols [256m, 256m+256)); the blocks
                # sit at x^T positions 4m and 4m+2 (contiguous slices)
                if a == 0:
                    ps_map[("q", m)] = projp.tile(
                        [128, PIECE], f32, tag="pj", name="psq"
                    )
                ps = ps_map[("q", m)]
                for s in range(2):
                    # both 128-col slots share one bank: a single
                    # accumulation group brackets all 8 matmuls
                    lo = 512 * m + 256 * s
                    nc.tensor.matmul(
                        ps[0:64, s * 128 : s * 128 + 128], wq_sb[:, a, :],
                        xT_sb[:, a, lo : lo + 128],
                        start=(a == 0 and s == 0), stop=(a == 3 and s == 1),
                    )

            def qp_copy(m):
                ps = ps_map.pop(("q", m))
                proj_copy("q", Qt[:, m * 256 : (m + 1) * 256], ps[0:64, 0:256])

            done = {}

            def q_items(m):
                if ("q", m) in done:
                    return []
                done[("q", m)] = True
                items = [lambda a=a: qp_mm(m, a) for a in range(4)]
                items.append(lambda: qp_copy(m))
                return items

            def prereq_items(j, p):
                """Projection items needed by unit (pair j, phase p)."""
                items = []
                if ("kv", j) not in done:
                    done[("kv", j)] = True
                    items += [lambda a=a: kv_mm(j, a) for a in range(4)]
                    items.append(lambda: kv_copy(j))
                    items.append(lambda: vt_mm(j, 0))
                    items.append(lambda: vt_mm(j, 1))
                    items.append(lambda: vt_copy(j))
                return items

            def epilogue(p, gi, bank):
                # q block 4p+gi just finished accumulating: normalize by the
                # ones-column sum, stage to SBUF, and DMA it out immediately
                # (per-block DMAs keep the queue warm so the final block's
                # DMA only pays descriptor+transfer, not the full init).
                jb = 4 * p + gi
                sl = gi if gi < 3 else 0
                rc = rcpp.tile([128, 1], f32, tag="rc")
                nc.vector.reciprocal(rc[:], bank[:, sl, H : H + 1])
                nc.vector.tensor_scalar_mul(
                    outsb[:, jb, :], bank[:, sl, 0:H], rc[:]
                )
                if p == NPIECES - 1 and gi == 3:
                    # last block rides the second HWDGE queue (ACT engine is
                    # done with exps) so the final two DMAs overlap
                    nc.scalar.dma_start(
                        out=out_r[:, jb, :], in_=outsb[:, jb, :]
                    )
                else:
                    nc.sync.dma_start(
                        out=out_r[:, jb, :], in_=outsb[:, jb, :]
                    )

            bgq = []          # (key, fn) FIFO of queued projection items
            pending = {}      # key -> items of that bundle still queued

            def pops(k):
                for _ in range(k):
                    if bgq:
                        key, fn = bgq.pop(0)
                        pending[key] -= 1
                        fn()

            def push(key, items, front=False):
                pending[key] = pending.get(key, 0) + len(items)
                wrapped = [(key, it) for it in items]
                if front:
                    # q bundles gate the next phase's first scores matmul:
                    # jump them ahead of any queued kv work
                    nq = sum(1 for kk, _ in bgq if kk[0] == "q")
                    bgq[nq:nq] = wrapped
                else:
                    bgq.extend(wrapped)

            def ensure(key):
                # drain the FIFO through this bundle so its items are
                # emitted before any consumer (emission-order safety)
                while pending.get(key, 0) > 0:
                    pops(1)

            def av_mms(prev_unit):
                # natural-orientation AV: av[qblk, 65] += P_k[qblk] @ Vn[k]
                # one matmul per (k tile, causal q block), free dim 65
                ptj, jj, pp, pav = prev_unit
                r = jj - 4 * pp  # local index of the diagonal q block
                # only one accumulation group may be open per PSUM bank, and
                # reads are only legal after the group's stop.  q blocks
                # {0,1} share one bank (group closes at unit 4p+1, so their
                # epilogues fire mid-phase) and {2,3} the other (closes at
                # the final unit).  start lazily zeroes the whole bank.
                pav012, pav3 = pav
                for gi in range(max(r, 0), 4):
                    bank, sl = (pav012, gi) if gi < 3 else (pav3, 0)
                    for s in range(2):
                        nc.tensor.matmul(
                            bank[:, sl, 0 : H + 1],
                            ptj[:, s, gi * 128 : (gi + 1) * 128],
                            Vn[:, 2 * jj + s, :],
                            start=(jj == 0 and gi in (0, 3) and s == 0),
                            stop=(s == 1 and jj == 4 * pp +
                                  (2 if gi < 3 else 3) and gi in (2, 3)),
                        )
                if jj == 4 * pp + 2:
                    # bank A (q blocks 0..2) closed: epilogues fire with one
                    # unit still to go, keeping only block 3 in the tail
                    for gg in range(3):
                        epilogue(pp, gg, pav012)
                elif jj == 4 * pp + 3:
                    epilogue(pp, 3, pav3)

            avq = []  # (pt, pair j, phase p, av tile) units awaiting AV;
            # AV trails scores by TWO units so its exp/mask (ACT/DVE) are
            # long finished when the in-order PE reaches the AV matmuls --
            # a 1-unit trail makes the PE wait out the exp latency every
            # unit.
            for p in range(NPIECES):
                # two full PSUM banks per phase (bank alignment keeps every
                # [*, gi%2, 0:65] accumulation region inside its bank):
                # q blocks {0,1} in one, {2,3} in the other
                av012 = avp.tile([128, 4, 128], f32, tag="av", name="av012")
                av3 = avp.tile([128, 4, 128], f32, tag="av", name="av3")
                av = (av012, av3)
                # this phase's Q halves MUST be emitted before unit 0 reads
                # the full piece (emission-order race otherwise); at phase 0
                # interleave kv(0) so its chain overlaps the 2nd chunk's DMA
                ensure(("q", 2 * p))
                for it in q_items(2 * p):
                    it()
                if p == 0:
                    for it in prereq_items(0, 0):
                        it()
                ensure(("q", 2 * p + 1))
                for it in q_items(2 * p + 1):
                    it()

                # queue next phase's Q + kv projection work
                if p + 1 < NPIECES:
                    push(("q", 2 * p + 2), q_items(2 * p + 2), front=True)
                    push(("q", 2 * p + 3), q_items(2 * p + 3), front=True)
                    for j in range(4 * p + 4, 4 * p + 8):
                        push(("kv", j), prereq_items(j, p + 1))
                for j in range(0, 4 * p + 4):
                    ensure(("kv", j))
                    for it in prereq_items(j, p):
                        it()
                    cl0 = 128 * j
                    rlo = max(cl0 - p * PIECE, 0)
                    # per-slot single-bank score tiles + per-slot exp: four
                    # half-units in flight in the same 4 PSUM banks, so the
                    # scores->exp->slot-recycle latency loop overlaps twice
                    # as deep and the exp engines can run concurrently
                    pt = ptp.tile([128, 2, PIECE], bf16, tag="pt")
                    eng = exp_engine(p, j)
                    for s in range(2):
                        k = 2 * j + s
                        ksl = slice(k * 128, (k + 1) * 128)
                        stx = stp.tile([128, PIECE], f32, tag="st")
                        nc.tensor.matmul(
                            stx[:, rlo:PIECE], KVt[0:64, ksl],
                            Qt[:, p * PIECE + rlo : (p + 1) * PIECE],
                            start=True, stop=True,
                        )
                        if eng == "act":
                            nc.scalar.activation(
                                pt[:, s, rlo:PIECE], stx[:, rlo:PIECE], EXP,
                                scale=inv_sqrt_c,
                            )
                        else:
                            ve = nc.gpsimd if eng == "pool" else nc.vector
                            ve.tensor_scalar(
                                pt[:, s, rlo:PIECE].bitcast(mybir.dt.int16),
                                stx[:, rlo:PIECE],
                                SCH_A, SCH_B,
                                mybir.AluOpType.mult, mybir.AluOpType.add,
                            )
                    if j >= 4 * p:
                        # this unit's causal diagonal (q block j) lies in
                        # this piece: multiplicative mask on its 128 cols
                        nc.vector.tensor_mul(
                            pt[:, :, rlo : rlo + 128],
                            pt[:, :, rlo : rlo + 128], msk2_sb[:],
                        )
                    avq.append((pt, j, p, av))
                    trail = 1 if (p == NPIECES - 1 and j >= 4 * p + 1) else 2
                    while len(avq) > trail:
                        av_mms(avq.pop(0))
                    pops(4)

            while avq:
                av_mms(avq.pop(0))
            while bgq:
                bgq.pop(0)()

    nc.compile()
    return nc


def _host_inputs(x, Wq, Wk, Wv):
    """Build the 8 per-core input maps (host-side layout prep only)."""
    # msk2[kr, s, qr] multiplicative keep-mask for the diagonal position
    # pair: slot 0 (own-parity key block == q block) is triangular for both
    # cores; slot 1 is the opposite parity: fully masked for par=0 (key
    # block above the diagonal), fully kept for par=1 (below).
    tri_keep = np.triu(np.ones((128, 128), np.float32))  # [kr, qr]: qr >= kr
    wq_r = Wq.reshape(4, 128, H).transpose(1, 0, 2).reshape(128, 4 * H)
    wkv = np.concatenate([Wk, Wv], axis=1)  # [C, 128]
    wkv_r = wkv.reshape(4, 128, 2 * H).transpose(1, 0, 2).reshape(128, 4 * 2 * H)
    wpack_par = []
    for par in (0, 1):
        cols = []
        for s in (0, 1):
            if s == 0:
                keep = tri_keep
            elif par == 0:
                keep = np.zeros((128, 128), np.float32)
            else:
                keep = np.ones((128, 128), np.float32)
            cols.append(keep)
        msk2 = np.concatenate(cols, axis=1)
        wpack_par.append(
            np.concatenate([wq_r, wkv_r, msk2], axis=1).astype(BF16)
        )
    in_maps = []
    for b in range(B):
        xb = x[b]
        xT = np.ascontiguousarray(xb.T).astype(BF16)  # [C, T]
        # par=1 core: swap adjacent 128-col blocks so own blocks sit at
        # even positions
        xTsw = np.ascontiguousarray(
            xT.reshape(C, NP, 2, 128)[:, :, ::-1, :].reshape(C, T)
        )
        for par in (0, 1):
            in_maps.append(
                {
                    "xT": xT if par == 0 else xTsw,
                    "wpack": wpack_par[par],
                }
            )
    return in_maps


def kernel(x, Wq, Wk, Wv, _want_trace=False):
    from concourse.bass_utils import run_bass_kernel_spmd

    x = np.asarray(x, dtype=np.float32)
    Wq = np.asarray(Wq, dtype=np.float32)
    Wk = np.asarray(Wk, dtype=np.float32)
    Wv = np.asarray(Wv, dtype=np.float32)

    if "nc" not in _PROGRAM_CACHE:
        _PROGRAM_CACHE["nc"] = build_program()
    nc = _PROGRAM_CACHE["nc"]

    in_maps = _host_inputs(x, Wq, Wk, Wv)
    res = run_bass_kernel_spmd(
        nc, in_maps, core_ids=list(range(NCORES)), trace=_want_trace
    )

    out = np.zeros((B, T, H), np.float32)
    for b in range(B):
        for par in (0, 1):
            r = res.results[2 * b + par]["out"]
            out[b].reshape(NK, 128, H)[par::2] = np.asarray(r, np.float32).reshape(
                NJ, 128, H
            )
    if _want_trace:
        return out, res
    return out

